# revision 1
# baseline (speedup 1.0000x reference)
"""Biquad IIR (DSVF) filter over x[512, 32768] on 8 trn2 NeuronCores.

Math: the filter y = lfilter(b, a, x) along time has poles strictly inside
the unit circle (g = tan(pi*sigmoid(.)/2) > 0, r = softplus(.) > 0), so the
impulse response h[k] decays geometrically.  Once |h[k]| falls below fp32
noise the IIR is exactly (to fp32) a K-tap FIR:

    y[t] = sum_{k<K} h[k] x[t-k]

With time on the SBUF partition axis in tiles of L=128, each output tile is
a sum of (D+1) Toeplitz matmuls against the current and D previous input
tiles, where D = ceil(K/L) - 1.  No recurrence survives on-device: every
tile is independent, so the 8 cores time-shard T with a D*L-sample halo.

Host side computes h (float64), the Toeplitz lhsT matrices, transposes x to
[T, B] so DMA loads [128 time, 512 batch] tiles are contiguous, and
transposes the result back.
"""

import math
import os

import numpy as np

B = 512
T = 32768
NCORES = 8
L = 128            # time-tile length == PE contract/partition dim
TPC = T // NCORES  # 4096 time steps per core
NT = TPC // L      # 32 output tiles per core

# matmul operand dtype: "float32" (exact, 4 cyc/row) or "float32r"
# (1 cyc/row at free-dim>=256, reduced-precision mode)
MM_DTYPE = os.environ.get("KERNEL_MM_DTYPE", "float32")
GROUP = 8          # PSUM banks used per matmul batch

_cache: dict = {}


def _coeffs(g, r, m_hp, m_bp, m_lp):
    g = float(np.asarray(g).reshape(()))
    r = float(np.asarray(r).reshape(()))
    m_hp = float(np.asarray(m_hp).reshape(()))
    m_bp = float(np.asarray(m_bp).reshape(()))
    m_lp = float(np.asarray(m_lp).reshape(()))
    gg = math.tan(math.pi * (1.0 / (1.0 + math.exp(-g))) / 2.0)
    rr = math.log1p(math.exp(r))
    g2 = gg * gg
    b0 = g2 * m_lp + gg * m_bp + m_hp
    b1 = 2.0 * g2 * m_lp - 2.0 * m_hp
    b2 = g2 * m_lp - gg * m_bp + m_hp
    a0 = g2 + 2.0 * rr * gg + 1.0
    a1 = 2.0 * g2 - 2.0
    a2 = g2 - 2.0 * rr * gg + 1.0
    return b0 / a0, b1 / a0, b2 / a0, a1 / a0, a2 / a0


def _impulse_response(b0, b1, b2, a1, a2, n):
    """h[0..n-1] of the DF2T biquad, float64."""
    h = np.empty(n, np.float64)
    z1 = z2 = 0.0
    for t in range(n):
        xt = 1.0 if t == 0 else 0.0
        y = b0 * xt + z1
        z1, z2 = b1 * xt - a1 * y + z2, b2 * xt - a2 * y
        h[t] = y
    return h


def _plan(b0, b1, b2, a1, a2):
    """Returns (D, [lhsT_0 .. lhsT_D]) where lhsT_d[i, j] = h[j + d*L - i]
    (tap from input sample i of the d-tiles-back input tile to output j)."""
    hmax_n = 64 * L
    h = _impulse_response(b0, b1, b2, a1, a2, hmax_n)
    scale = np.max(np.abs(h))
    sig = np.nonzero(np.abs(h) > 1e-10 * scale)[0]
    last = int(sig[-1]) if len(sig) else 0
    if last >= hmax_n - 1:
        raise ValueError("impulse response decays too slowly for FIR plan")
    D = max(1, -(-(last + 1) // L) - 1)
    ws = []
    i = np.arange(L)[:, None]
    j = np.arange(L)[None, :]
    for d in range(D + 1):
        k = j + d * L - i
        w = np.where((k >= 0) & (k < hmax_n), h[np.clip(k, 0, hmax_n - 1)], 0.0)
        ws.append(np.ascontiguousarray(w, np.float32))
    return D, ws


def _build(D, mm_dtype_name):
    import concourse.bacc as bacc
    import concourse.mybir as mybir
    import concourse.tile as tile
    from contextlib import ExitStack

    f32 = mybir.dt.float32
    mm_dt = getattr(mybir.dt, mm_dtype_name)

    nc = bacc.Bacc("TRN2", target_bir_lowering=False, debug=False,
                   enable_asserts=False)
    xt = nc.dram_tensor("xt", [TPC + D * L, B], f32, kind="ExternalInput").ap()
    wds = [
        nc.dram_tensor(f"w{d}", [L, L], f32, kind="ExternalInput").ap()
        for d in range(D + 1)
    ]
    yt = nc.dram_tensor("yt", [TPC, B], f32, kind="ExternalOutput").ap()

    with ExitStack() as ctx:
        tc = ctx.enter_context(tile.TileContext(nc))
        wpool = ctx.enter_context(tc.tile_pool(name="wpool", bufs=1))
        xpool = ctx.enter_context(tc.tile_pool(name="xpool", bufs=14))
        ypool = ctx.enter_context(tc.tile_pool(name="ypool", bufs=6))
        pspool = ctx.enter_context(
            tc.tile_pool(name="pspool", bufs=GROUP, space="PSUM"))

        wts = []
        for d in range(D + 1):
            wtile = wpool.tile([L, L], f32, name=f"wt{d}", tag=f"wt{d}")
            nc.sync.dma_start(wtile[:], wds[d][:])
            wts.append(wtile)

        xts = []
        for c in range(NT + D):
            xtile = xpool.tile([L, B], f32, name=f"xt{c}", tag="xtile")
            nc.sync.dma_start(xtile[:], xt[c * L:(c + 1) * L, :])
            xts.append(xtile)

        for g0 in range(0, NT, GROUP):
            n = min(GROUP, NT - g0)
            pss = [
                pspool.tile([L, B], f32, name=f"ps{g0 + k}", tag="ps")
                for k in range(n)
            ]
            # weight-major: all matmuls sharing a stationary operand are
            # adjacent, accumulating across D+1 passes into n PSUM banks
            for d in range(D, -1, -1):
                for k in range(n):
                    o = g0 + k
                    nc.tensor.matmul(
                        pss[k][:],
                        wts[d][:].bitcast(mm_dt),
                        xts[o + D - d][:].bitcast(mm_dt),
                        start=(d == D),
                        stop=(d == 0),
                    )
            for k in range(n):
                o = g0 + k
                ytile = ypool.tile([L, B], f32, name=f"yt{o}", tag="ytile")
                # alternate drain between DVE and ACT so neither bottlenecks
                if k % 2 == 0:
                    nc.vector.tensor_copy(ytile[:], pss[k][:])
                else:
                    nc.scalar.copy(ytile[:], pss[k][:])
                nc.sync.dma_start(yt[o * L:(o + 1) * L, :], ytile[:])
    nc.compile()
    return nc


_last_results = None


def kernel(x, g, r, m_hp, m_bp, m_lp):
    global _last_results
    from concourse.bass_utils import run_bass_kernel_spmd

    b0, b1, b2, a1, a2 = _coeffs(g, r, m_hp, m_bp, m_lp)
    key = (round(b0, 12), round(b1, 12), round(b2, 12),
           round(a1, 12), round(a2, 12))
    if key not in _cache:
        D, ws = _plan(b0, b1, b2, a1, a2)
        nc = _build(D, MM_DTYPE)
        _cache[key] = (D, ws, nc)
    D, ws, nc = _cache[key]

    x = np.asarray(x, np.float32)
    xt_pad = np.zeros((T + D * L, B), np.float32)
    xt_pad[D * L:] = x.T

    in_maps = []
    for i in range(NCORES):
        m = {"xt": np.ascontiguousarray(xt_pad[i * TPC:(i + 1) * TPC + D * L])}
        for d in range(D + 1):
            m[f"w{d}"] = ws[d]
        in_maps.append(m)

    res = run_bass_kernel_spmd(
        nc, in_maps, core_ids=list(range(NCORES)),
        trace=bool(int(os.environ.get("KERNEL_TRACE", "0"))),
    )
    _last_results = res
    yt = np.concatenate([r["yt"] for r in res.results], axis=0)
    return np.ascontiguousarray(yt.T)


# revision 4
# speedup vs baseline: 1.1681x; 1.1681x over previous
"""Biquad IIR (DSVF) filter over x[512, 32768] on 8 trn2 NeuronCores.

Math: the filter y = lfilter(b, a, x) along time has poles strictly inside
the unit circle (g = tan(pi*sigmoid(.)/2) > 0, r = softplus(.) > 0), so the
impulse response h[k] decays geometrically.  Once |h[k]| falls below fp32
noise the IIR is exactly (to fp32) a K-tap FIR:

    y[t] = sum_{k<K} h[k] x[t-k]

With time on the SBUF partition axis in tiles of L=128, each output tile is
a sum of (D+1) Toeplitz matmuls against the current and D previous input
tiles, where D = ceil(K/L) - 1.  No recurrence survives on-device: every
tile is independent, so the 8 cores time-shard T with a D*L-sample halo.

Host side computes h (float64), the Toeplitz lhsT matrices, transposes x to
[T, B] so DMA loads [128 time, 512 batch] tiles are contiguous, and
transposes the result back.
"""

import math
import os

import numpy as np

B = 512
T = 32768
NCORES = 8
L = 128            # time-tile length == PE contract/partition dim
TPC = T // NCORES  # 4096 time steps per core
NT = TPC // L      # 32 output tiles per core

# matmul operand dtype: "float32" (exact, 4 cyc/row) or "float32r"
# (1 cyc/row at free-dim>=256, reduced-precision mode)
MM_DTYPE = os.environ.get("KERNEL_MM_DTYPE", "float32")
GROUP = 8          # PSUM banks used per matmul batch

_cache: dict = {}


def _coeffs(g, r, m_hp, m_bp, m_lp):
    g = float(np.asarray(g).reshape(()))
    r = float(np.asarray(r).reshape(()))
    m_hp = float(np.asarray(m_hp).reshape(()))
    m_bp = float(np.asarray(m_bp).reshape(()))
    m_lp = float(np.asarray(m_lp).reshape(()))
    gg = math.tan(math.pi * (1.0 / (1.0 + math.exp(-g))) / 2.0)
    rr = math.log1p(math.exp(r))
    g2 = gg * gg
    b0 = g2 * m_lp + gg * m_bp + m_hp
    b1 = 2.0 * g2 * m_lp - 2.0 * m_hp
    b2 = g2 * m_lp - gg * m_bp + m_hp
    a0 = g2 + 2.0 * rr * gg + 1.0
    a1 = 2.0 * g2 - 2.0
    a2 = g2 - 2.0 * rr * gg + 1.0
    return b0 / a0, b1 / a0, b2 / a0, a1 / a0, a2 / a0


def _impulse_response(b0, b1, b2, a1, a2, n):
    """h[0..n-1] of the DF2T biquad, float64."""
    h = np.empty(n, np.float64)
    z1 = z2 = 0.0
    for t in range(n):
        xt = 1.0 if t == 0 else 0.0
        y = b0 * xt + z1
        z1, z2 = b1 * xt - a1 * y + z2, b2 * xt - a2 * y
        h[t] = y
    return h


def _plan(b0, b1, b2, a1, a2):
    """Returns (D, [lhsT_0 .. lhsT_D]) where lhsT_d[i, j] = h[j + d*L - i]
    (tap from input sample i of the d-tiles-back input tile to output j)."""
    hmax_n = 64 * L
    h = _impulse_response(b0, b1, b2, a1, a2, hmax_n)
    scale = np.max(np.abs(h))
    sig = np.nonzero(np.abs(h) > 1e-10 * scale)[0]
    last = int(sig[-1]) if len(sig) else 0
    if last >= hmax_n - 1:
        raise ValueError("impulse response decays too slowly for FIR plan")
    D = max(1, -(-(last + 1) // L) - 1)
    ws = []
    i = np.arange(L)[:, None]
    j = np.arange(L)[None, :]
    for d in range(D + 1):
        k = j + d * L - i
        w = np.where((k >= 0) & (k < hmax_n), h[np.clip(k, 0, hmax_n - 1)], 0.0)
        ws.append(np.ascontiguousarray(w, np.float32))
    return D, ws


def _build(D, mm_dtype_name):
    import concourse.bacc as bacc
    import concourse.mybir as mybir
    import concourse.tile as tile
    from contextlib import ExitStack

    f32 = mybir.dt.float32
    mm_dt = getattr(mybir.dt, mm_dtype_name)

    nc = bacc.Bacc("TRN2", target_bir_lowering=False, debug=False,
                   enable_asserts=False)
    xt = nc.dram_tensor("xt", [TPC + D * L, B], mm_dt, kind="ExternalInput").ap()
    wds = [
        nc.dram_tensor(f"w{d}", [L, L], mm_dt, kind="ExternalInput").ap()
        for d in range(D + 1)
    ]
    yt = nc.dram_tensor("yt", [TPC, B], f32, kind="ExternalOutput").ap()

    with ExitStack() as ctx:
        tc = ctx.enter_context(tile.TileContext(nc))
        wpool = ctx.enter_context(tc.tile_pool(name="wpool", bufs=1))
        xpool = ctx.enter_context(tc.tile_pool(name="xpool", bufs=14))
        ypool = ctx.enter_context(tc.tile_pool(name="ypool", bufs=6))
        pspool = ctx.enter_context(
            tc.tile_pool(name="pspool", bufs=GROUP, space="PSUM"))

        wts = []
        for d in range(D + 1):
            wtile = wpool.tile([L, L], mm_dt, name=f"wt{d}", tag=f"wt{d}")
            nc.sync.dma_start(wtile[:], wds[d][:])
            wts.append(wtile)

        xts = []
        for c in range(NT + D):
            xtile = xpool.tile([L, B], mm_dt, name=f"xt{c}", tag="xtile")
            nc.sync.dma_start(xtile[:], xt[c * L:(c + 1) * L, :])
            xts.append(xtile)

        for g0 in range(0, NT, GROUP):
            n = min(GROUP, NT - g0)
            pss = [
                pspool.tile([L, B], f32, name=f"ps{g0 + k}", tag="ps")
                for k in range(n)
            ]
            # weight-major: all matmuls sharing a stationary operand are
            # adjacent, accumulating across D+1 passes into n PSUM banks
            for d in range(D, -1, -1):
                for k in range(n):
                    o = g0 + k
                    nc.tensor.matmul(
                        pss[k][:],
                        wts[d][:],
                        xts[o + D - d][:],
                        start=(d == D),
                        stop=(d == 0),
                    )
            for k in range(n):
                o = g0 + k
                ytile = ypool.tile([L, B], f32, name=f"yt{o}", tag="ytile")
                # alternate drain between DVE and ACT so neither bottlenecks
                if k % 2 == 0:
                    nc.vector.tensor_copy(ytile[:], pss[k][:])
                else:
                    nc.scalar.copy(ytile[:], pss[k][:])
                nc.sync.dma_start(yt[o * L:(o + 1) * L, :], ytile[:])
    nc.compile()
    return nc


_last_results = None


def kernel(x, g, r, m_hp, m_bp, m_lp):
    global _last_results
    from concourse.bass_utils import run_bass_kernel_spmd

    b0, b1, b2, a1, a2 = _coeffs(g, r, m_hp, m_bp, m_lp)
    key = (round(b0, 12), round(b1, 12), round(b2, 12),
           round(a1, 12), round(a2, 12))
    if key not in _cache:
        D, ws = _plan(b0, b1, b2, a1, a2)
        nc = _build(D, MM_DTYPE)
        _cache[key] = (D, ws, nc)
    D, ws, nc = _cache[key]

    x = np.asarray(x, np.float32)
    xt_pad = np.zeros((T + D * L, B), np.float32)
    xt_pad[D * L:] = x.T

    in_maps = []
    for i in range(NCORES):
        m = {"xt": np.ascontiguousarray(xt_pad[i * TPC:(i + 1) * TPC + D * L])}
        for d in range(D + 1):
            m[f"w{d}"] = ws[d]
        in_maps.append(m)

    res = run_bass_kernel_spmd(
        nc, in_maps, core_ids=list(range(NCORES)),
        trace=bool(int(os.environ.get("KERNEL_TRACE", "0"))),
    )
    _last_results = res
    yt = np.concatenate([r["yt"] for r in res.results], axis=0)
    return np.ascontiguousarray(yt.T)
